# revision 1
# baseline (speedup 1.0000x reference)
"""Linear-CKA map kernel for Trainium2 (8 NeuronCores, SPMD, no collectives).

Math: for activations X[l] ([B, D] per layer), the reference computes
Gram matrices G_l = X_l X_l^T, double-centers them (Gc = H G H), and
hsic[i,j] = <Gc_i, Gc_j>, cka = hsic / sqrt(diag outer).

We use the expansion (H idempotent, G symmetric):
    hsic_ij = S_ij - (2/B) * T_ij + u_i u_j / B^2
      S_ij = <G_i, G_j>
      rowsum_l[b] = sum_c G_l[b, c] = X_l[b, :] . s_l,  s_l = sum_b X_l[b, :]
      T_ij = sum_b rowsum_i[b] rowsum_j[b]
      u_l  = s_l . s_l

Sharding: the Gram is symmetric, so only its block upper triangle is
needed.  With 16x16 blocks of [128, 128], core k computes the cyclic
cover blocks (bi, (bi + t) mod 16), t = 0..8, for its two block rows
bi in {2k, 2k+1} -- a perfectly uniform SPMD program (18 blocks per
core; every unordered block pair is covered once, except the t=8
antipodal blocks which two cores split).  Weight classes: t=0 diag
blocks count once, t=1..7 twice, t=8 once; the three classes accumulate
into separate [24,24] PSUM groups and the host combines g0 + 2*g1 + g2.

Per-core input: xr = X^T columns [2k*128 : 2k*128+1280] (mod B), in
fp8-e4m3 (CKA is a normalized statistic, so input quantization noise
stays ~1e-4 in the output) -- both Gram matmul operands come from this
one 63 MB slice, read exactly once, and the matmuls run in DoubleRow
mode (256-deep contraction, 2 fp8 MACs/cell/cycle).  S reduces in two
overlapped pieces: pairs within the first LA layers stream through the
otherwise-idle VectorE *during* the DMA-bound main loop (they unlock
as each layer's Gram lands), while TensorE finishes the rest in a short
tail: the PSUM->SBUF copies interleave layers ([b, c, layer] bf16),
then [128b, 4x24] x [128b, 4x14] matmuls (4 Gram columns per call, rhs
restricted to the remaining LB layers) accumulate S in PSUM; the host
keeps the diagonal blocks.  Partial S is the only device output,
summed on the host -- no device collective at all.  The O(L*B*D)
row-sum statistics T and u are computed on the host (0.02% of FLOPs).
"""

import numpy as np
import ml_dtypes

L, B, D = 24, 2048, 2048
NCORES = 8
P = 128
NBLK = B // P               # 16 block rows/cols
JT = D // (2 * P)           # 8 double-row contraction tiles (256 deep each)
JG = 2                      # j-tiles fetched per rhs DMA
NT = 9                      # cyclic block offsets t = 0..8 per block row
NR = 2                      # block rows per core
NXC = NT + NR - 1           # 10 column chunks staged per core
WC = NT * P                 # 1152 Gram columns per block row
LA = 10                     # layers whose intra-pairs reduce on idle VectorE
LB = L - LA                 # layers handled by the TensorE S-stage rhs
NPA = LA * (LA + 1) // 2    # VectorE pair count

_NC_CACHE = {}


def _build():
    if "nc" in _NC_CACHE:
        return _NC_CACHE["nc"]
    import concourse.bass as bass
    from concourse import bacc, mybir, tile

    f32 = mybir.dt.float32
    bf16 = mybir.dt.bfloat16
    fp8 = mybir.dt.float8e4
    DR = mybir.MatmulPerfMode.DoubleRow
    Act = mybir.ActivationFunctionType
    Alu = mybir.AluOpType

    nc = bacc.Bacc("TRN2", target_bir_lowering=False, debug=False)

    # xr is host-packed to exactly match the SBUF staging tiles: one fully
    # linear [P, JG, 2, 1280] block per (layer, jg) DMA
    xr = nc.dram_tensor(
        "xr", [L, JT // JG, P, JG, 2, NXC * P], fp8, kind="ExternalInput"
    )
    s_out = nc.dram_tensor("s_out", [3, 4 * L, 4 * LB], f32, kind="ExternalOutput")
    a_out = nc.dram_tensor("a_out", [1, NR * 3 * NPA], f32, kind="ExternalOutput")

    with tile.TileContext(nc) as tc:
        with (
            tc.tile_pool(name="gb", bufs=1) as gbpool,
            tc.tile_pool(name="rt", bufs=5) as rtpool,
            tc.tile_pool(name="small", bufs=1) as smallpool,
            tc.tile_pool(name="psum", bufs=2, space=bass.MemorySpace.PSUM) as psumpool,
            tc.tile_pool(name="psumS", bufs=1, space=bass.MemorySpace.PSUM) as psSpool,
        ):
            # persistent SBUF: interleaved Gram store [b, c, layer] per row
            gbig16 = [
                gbpool.tile([P, P, L], bf16, tag=f"Gb16{r}", name=f"Gb16{r}")
                for r in range(NR)
            ]
            gbig8 = [
                gbpool.tile([P, WC - P, L], fp8, tag=f"Gb8{r}", name=f"Gb8{r}")
                for r in range(NR)
            ]
            # NOTE: matmul start=True pending-zeroes its whole PSUM bank, so
            # each bank may host only ONE accumulation group at a time (the
            # 512/512/128 Gram split below is bank-aligned for this reason,
            # and the S classes accumulate sequentially with copies between).
            QW = 4 * L          # S-stage stationary width (4 Gram columns)
            QN = 4 * LB         # S-stage moving width (B layers only)
            ptS = psSpool.tile([P, 3 * QN], f32, tag="psS")
            # VectorE A-block pair accumulators, (r, class)-sliced
            pairacc = smallpool.tile([P, NR * 3 * NPA], f32, tag="pairacc")
            sttout = smallpool.tile([P, WC], bf16, tag="sttout")

            for l in range(L):
                pt = [
                    psumpool.tile([P, WC], f32, tag="pm", name=f"pm{r}")
                    for r in range(NR)
                ]
                for jg in range(JT // JG):
                    rt = rtpool.tile([P, JG, 2, NXC * P], fp8, tag="rt")
                    nc.sync.dma_start(rt[:, :, :, :], xr[l, jg])
                    for jj in range(JG):
                        j = jg * JG + jj
                        for r in range(NR):
                            lhs = rt[:, jj, :, r * P : (r + 1) * P]
                            # 9 cyclic blocks = contiguous 1152 rhs columns,
                            # split 512/512/128 on PSUM bank boundaries
                            for c0, cw in ((0, 512), (512, 512), (1024, 128)):
                                nc.tensor.matmul(
                                    pt[r][:, c0 : c0 + cw],
                                    lhsT=lhs,
                                    rhs=rt[:, jj, :, r * P + c0 : r * P + c0 + cw],
                                    start=(j == 0),
                                    stop=(j == JT - 1),
                                    perf_mode=DR,
                                )
                if l == L - 1:
                    # issue both small diag-block copies first so the g0
                    # quads unlock while the big scaled fp8 copies stream
                    # (VectorE is still draining its pair backlog here)
                    nc.scalar.copy(gbig16[0][:, :, l], pt[0][:, 0:P])
                    nc.scalar.copy(gbig16[1][:, :, l], pt[1][:, 0:P])
                    nc.scalar.mul(gbig8[0][:, :, l], pt[0][:, P:WC], 0.5)
                    nc.scalar.mul(gbig8[1][:, :, l], pt[1][:, P:WC], 0.5)
                else:
                    for r in range(NR):
                        nc.scalar.copy(gbig16[r][:, :, l], pt[r][:, 0:P])
                        nc.scalar.mul(gbig8[r][:, :, l], pt[r][:, P:WC], 0.5)
                # A-block pairs (i, l), i <= l < LA reduce on the otherwise
                # idle VectorE while the DMA-bound main loop continues; the
                # three weight classes accumulate into separate columns
                if l < LA:
                    for i in range(l + 1):
                        p = l * (l + 1) // 2 + i
                        for r in range(NR):
                            for cls, (st, lo, hi) in enumerate(
                                ((0, 0, P), (1, 0, 7 * P), (1, 7 * P, 8 * P))
                            ):
                                gsrc = gbig16[r] if st == 0 else gbig8[r]
                                nc.vector.scalar_tensor_tensor(
                                    out=sttout[:, lo:hi],
                                    in0=gsrc[:, lo:hi, i],
                                    scalar=1.0,
                                    in1=gsrc[:, lo:hi, l],
                                    op0=Alu.mult,
                                    op1=Alu.mult,
                                    accum_out=pairacc[
                                        :, (r * 3 + cls) * NPA + p : (r * 3 + cls) * NPA + p + 1
                                    ],
                                )

            # partition-reduce the VectorE pair accumulators on idle GpSimd
            asum = smallpool.tile([P, NR * 3 * NPA], f32, tag="asum")
            nc.gpsimd.tensor_reduce(
                asum[0:1, :], pairacc[:, :], axis=mybir.AxisListType.C, op=Alu.add
            )
            nc.sync.dma_start(a_out[:, :], asum[0:1, :])

            # S-stage on TensorE: [128b, 96] x [128b, 96] matmuls, 4 Gram
            # columns per call (gbig's [c, l] layout is contiguous, so 4
            # columns = one flat 96-wide operand).  Only the 4 diagonal
            # [24, 24] blocks of each [96, 96] product matter; the host
            # discards the cross-column junk.  Weight classes accumulate
            # SEQUENTIALLY (one live group in the shared PSUM bank) and are
            # copied out before the next class's start=True re-marks the bank.
            # lhsT spans all L layers (out rows cover every i), rhs spans only
            # the LB "B" layers -- the A-block intra-pairs came from VectorE
            sall = smallpool.tile([P, 3 * QN], f32, tag="sall")
            # class 0: bf16 quads on the diag-block store; classes 1/2:
            # fp8 DoubleRow octs (pair dim = c-offset 0..3 vs 4..7) on the
            # off-diag store -- the pair dim contracts away so the [96, 56]
            # extraction is identical
            cls_src = {0: (0, 0, P, 4), 1: (1, 0, 7 * P, 8), 2: (1, 7 * P, 8 * P, 8)}
            for cls in range(3):
                st, lo, hi, step = cls_src[cls]
                for r in range(NR):
                    for c in range(lo, hi, step):
                        if st == 0:
                            nc.tensor.matmul(
                                ptS[0:QW, cls * QN : (cls + 1) * QN],
                                lhsT=gbig16[r][:, c : c + 4, :],
                                rhs=gbig16[r][:, c : c + 4, LA:L],
                                start=(r == 0 and c == lo),
                                stop=(r == NR - 1 and c == hi - step),
                            )
                        else:
                            nc.tensor.matmul(
                                ptS[0:QW, cls * QN : (cls + 1) * QN],
                                lhsT=gbig8[r][:, c : c + 8, :].rearrange(
                                    "p (i x) l -> p i (x l)", i=2
                                ),
                                rhs=gbig8[r][:, c : c + 8, LA:L].rearrange(
                                    "p (i x) l -> p i x l", i=2
                                ),
                                start=(r == 0 and c == lo),
                                stop=(r == NR - 1 and c == hi - step),
                                perf_mode=DR,
                            )
                nc.scalar.copy(
                    sall[0:QW, cls * QN : (cls + 1) * QN],
                    ptS[0:QW, cls * QN : (cls + 1) * QN],
                )
                nc.sync.dma_start(
                    s_out[cls], sall[0:QW, cls * QN : (cls + 1) * QN]
                )

    nc.compile()
    _NC_CACHE["nc"] = nc
    return nc


def _run(activations, trace=False):
    from concourse.bass_utils import run_bass_kernel_spmd

    x = np.asarray(activations, dtype=np.float32)
    assert x.shape == (L, B, D)
    xt_np = np.ascontiguousarray(x.transpose(0, 2, 1)).astype(ml_dtypes.float8_e4m3)
    s_star = xt_np.astype(np.float64).sum(axis=2)  # [L, D], exact sum of fp8 X

    in_maps = []
    for c in range(NCORES):
        lo = NR * c * P
        rolled = np.concatenate([xt_np[:, :, lo:], xt_np[:, :, :lo]], axis=2)[
            :, :, : NXC * P
        ]
        # pack to the SBUF staging layout: [L, jg, p, jj, i, n] so each
        # (layer, jg) DMA is one fully contiguous block
        packed = np.ascontiguousarray(
            rolled.reshape(L, JT // JG, JG, 2, P, NXC * P).transpose(0, 1, 4, 2, 3, 5)
        )
        in_maps.append({"xr": packed})
    nc = _build()
    try:
        res = run_bass_kernel_spmd(
            nc, in_maps, core_ids=list(range(NCORES)), trace=trace
        )
    except Exception:
        # transient NRT_EXEC_UNIT_UNRECOVERABLE device states have been
        # observed to clear on the next attempt
        import time

        time.sleep(5)
        res = run_bass_kernel_spmd(
            nc, in_maps, core_ids=list(range(NCORES)), trace=trace
        )

    S = np.zeros((L, L), dtype=np.float64)
    for c in range(NCORES):
        # TensorE part: [3, 4*L, 4*LB] quad blocks, diagonal-in-quad only
        g = res.results[c]["s_out"].astype(np.float64).reshape(3, 4, L, 4, LB)
        gd = [sum(g[i, d, :, d, :] for d in range(4)) for i in range(3)]
        Sc = np.zeros((L, L))
        Sc[:, LA:] = gd[0] + 8.0 * gd[1] + 4.0 * gd[2]
        Sc[LA:, :LA] = Sc[:LA, LA:].T
        # VectorE part: A-block pairs, (r, class)-sliced partials
        a = res.results[c]["a_out"].astype(np.float64).reshape(NR, 3, NPA)
        av = a.sum(axis=0)
        pa = av[0] + 8.0 * av[1] + 4.0 * av[2]
        for l in range(LA):
            for i in range(l + 1):
                v = pa[l * (l + 1) // 2 + i]
                Sc[i, l] = v
                Sc[l, i] = v
        S += Sc

    # row-sum statistics are O(L*B*D) -- computed host-side on the same
    # quantized values the device consumed
    xq = xt_np.astype(np.float32)                  # [L, D, B]
    rowsum = np.einsum("ldb,ld->lb", xq, s_star.astype(np.float32))
    T = np.einsum("ib,jb->ij", rowsum, rowsum, dtype=np.float64)
    u = np.einsum("ld,ld->l", s_star, s_star)
    hsic = S - (2.0 / B) * T + np.outer(u, u) / (B * B)
    norms = np.sqrt(np.diagonal(hsic))
    cka = hsic / (norms[:, None] * norms[None, :])
    return cka.astype(np.float32), res


def kernel(activations):
    cka, _ = _run(activations, trace=False)
    return cka


def run_traced(activations):
    return _run(activations, trace=True)



# revision 2
# speedup vs baseline: 1.1896x; 1.1896x over previous
"""Linear-CKA map kernel for Trainium2 (8 NeuronCores, SPMD, no collectives).

Math: for activations X[l] ([B, D] per layer), the reference computes
Gram matrices G_l = X_l X_l^T, double-centers them (Gc = H G H), and
hsic[i,j] = <Gc_i, Gc_j>, cka = hsic / sqrt(diag outer).

We use the expansion (H idempotent, G symmetric):
    hsic_ij = S_ij - (2/B) * T_ij + u_i u_j / B^2
      S_ij = <G_i, G_j>
      rowsum_l[b] = sum_c G_l[b, c] = X_l[b, :] . s_l,  s_l = sum_b X_l[b, :]
      T_ij = sum_b rowsum_i[b] rowsum_j[b]
      u_l  = s_l . s_l

Sharding: the Gram is symmetric over 16x16 blocks of [128, 128]; every
unordered block pair must be co-resident on some core.  A covering
design with 7 column-blocks per core (the information-theoretic floor
at this granularity: 6-block coverings do not exist, by a Fisher /
intersecting-family argument) brings the per-core HBM read down to
L * D * 896 fp8 bytes = 44 MB, vs 63 MB for the baseline 10-block
cyclic cover.  All cores run the SAME program over 7 SBUF "slots"; a
per-core slot permutation SIGMA (found by annealing over an ILP cover)
maps slots to physical blocks so that the fixed slot-pair work list
covers all 136 physical pairs:

  slot-pairs = 2 self pairs (0,0),(1,1) + all 18 cross pairs (a,b)
  with hub a <= 3 -- four "fans" with contiguous partner ranges so
  each PSUM bank hosts exactly one accumulation group:
     F0 = (0 x slots 0..6)   896 cols   2 banks
     F1 = (1 x slots 1..6)   768 cols   2 banks
     F2 = (2 x slots 3..6)   512 cols   1 bank
     F3 = (3 x slots 4..6)   384 cols   1 bank
  The blocks at slots 0,1 across the 8 cores partition all 16 blocks,
  so the two self pairs compute each Gram diagonal block exactly once.

Per layer the fans run in two staggered groups A={F0,F3}, B={F1,F2}
(3 PSUM banks each): A's PSUM->SBUF copies drain on ScalarE while B's
matmuls run, so TensorE never stalls on bank reuse.  Gram blocks are
stored interleaved [b, c, layer] (diag in bf16, off-diag in fp8 at 0.5
scale), and a short TensorE tail reduces each stored slot-pair against
itself over layers ([128b, 4c x 24l] x [128b, 4c x 24l] quad/oct
matmuls) into per-slot-pair [96, 96] tiles whose quad-diagonal 24x24
blocks the host extracts.  The host de-duplicates redundantly covered
pairs with a precomputed (core, slot-pair) ownership map and adds the
O(L*B*D) row-sum statistics T and u.  No device collective at all.
"""

import numpy as np
import ml_dtypes

L, B, D = 24, 2048, 2048
NCORES = 8
P = 128
NS = 7                      # column-block slots per core
JT = 8                      # 256-deep DoubleRow contraction tiles
JG = 2                      # j-tiles fetched per rhs DMA
W = NS * P                  # 896 packed columns per core

# fans: (hub slot, first partner slot, #partners); bank-aligned matmul
# splits of each fan's PSUM tile are derived below
FANS = [(0, 0, 7), (1, 1, 6), (2, 3, 4), (3, 4, 3)]
# cross slot-pairs in g8 storage order: F0 partners 1..6, F3, F1
# partners 2..6, F2 (the hub-0/hub-1 self pairs live in gd0/gd1)
CROSSQ = (
    [(0, b) for b in range(1, 7)]
    + [(3, b) for b in range(4, 7)]
    + [(1, b) for b in range(2, 7)]
    + [(2, b) for b in range(3, 7)]
)
NQ = len(CROSSQ)            # 18
NPAIR = NQ + 2              # + the two self pairs

# SIGMA[k][s] = physical block held in slot s on core k (annealed so the
# fixed slot-pair list covers all 136 block pairs and slots {0,1}
# partition the 16 diagonal blocks)
SIGMA = [
    [15, 8, 12, 11, 1, 7, 6],
    [2, 11, 14, 8, 9, 10, 3],
    [4, 13, 5, 0, 8, 12, 11],
    [3, 10, 13, 9, 12, 7, 15],
    [0, 7, 6, 5, 10, 3, 9],
    [5, 14, 0, 2, 15, 7, 1],
    [1, 9, 4, 10, 7, 3, 15],
    [12, 6, 13, 4, 1, 2, 14],
]

# host-side dedup: first core covering a physical pair owns it
_OWNER_W = np.zeros((NCORES, NQ), dtype=np.float64)
_seen = {}
for _k in range(NCORES):
    for _q, (_a, _b) in enumerate(CROSSQ):
        _u, _v = SIGMA[_k][_a], SIGMA[_k][_b]
        _pp = (min(_u, _v), max(_u, _v))
        if _pp not in _seen:
            _seen[_pp] = True
            # weight 2 for Gram symmetry x4 to undo the 0.5 fp8 store scale
            _OWNER_W[_k][_q] = 8.0
assert len(_seen) == 120

_NC_CACHE = {}


def _build():
    if "nc" in _NC_CACHE:
        return _NC_CACHE["nc"]
    import concourse.bass as bass
    from concourse import bacc, mybir, tile

    f32 = mybir.dt.float32
    bf16 = mybir.dt.bfloat16
    fp8 = mybir.dt.float8e4
    DR = mybir.MatmulPerfMode.DoubleRow

    nc = bacc.Bacc("TRN2", target_bir_lowering=False, debug=False)

    # host-packed to match the SBUF staging tiles: one fully linear
    # [P, JG, 2, W] block per (layer, jg) DMA
    xr = nc.dram_tensor("xr", [L, JT // JG, P, JG, 2, W], fp8, kind="ExternalInput")
    s_out = nc.dram_tensor("s_out", [4 * L, NPAIR * 4 * L], f32, kind="ExternalOutput")

    QW = 4 * L  # 96: quad/oct S-stage operand width (4 Gram cols x L layers)

    with tile.TileContext(nc) as tc:
        with (
            tc.tile_pool(name="gb", bufs=1) as gbpool,
            tc.tile_pool(name="rt", bufs=8) as rtpool,
            tc.tile_pool(name="psum", bufs=1, space=bass.MemorySpace.PSUM) as pfpool,
            tc.tile_pool(name="psumS", bufs=2, space=bass.MemorySpace.PSUM) as psSpool,
        ):
            # persistent SBUF Gram store, interleaved [b, c, layer]
            gd0 = gbpool.tile([P, P, L], bf16, tag="gd0", name="gd0")
            gd1 = gbpool.tile([P, P, L], bf16, tag="gd1", name="gd1")
            g8 = gbpool.tile([P, NQ * P, L], fp8, tag="g8", name="g8")
            sS = gbpool.tile([QW, NPAIR * QW], f32, tag="sS", name="sS")

            # one PSUM tile per fan; a fan's matmuls split on its tile's
            # bank boundaries so each bank hosts ONE accumulation group
            pf0 = pfpool.tile([P, 7 * P], f32, tag="pf0", name="pf0")
            pf1 = pfpool.tile([P, 6 * P], f32, tag="pf1", name="pf1")
            pf2 = pfpool.tile([P, 4 * P], f32, tag="pf2", name="pf2")
            pf3 = pfpool.tile([P, 3 * P], f32, tag="pf3", name="pf3")

            def fan_matmuls(rt, jj, st, sp, pf, hub, p0, np_):
                lhs = rt[:, jj, :, hub * P : (hub + 1) * P]
                # split the fan's [p0, p0+np_) partner range on the PSUM
                # bank (512 f32) boundaries of pf
                c = 0
                while c < np_ * P:
                    cw = min(512 - c % 512, np_ * P - c)
                    nc.tensor.matmul(
                        pf[:, c : c + cw],
                        lhsT=lhs,
                        rhs=rt[:, jj, :, p0 * P + c : p0 * P + c + cw],
                        start=st,
                        stop=sp,
                        perf_mode=DR,
                    )
                    c += cw

            for l in range(L):
                rts = []
                for jg in range(JT // JG):
                    rt = rtpool.tile([P, JG, 2, W], fp8, tag="rt", name="rt")
                    nc.sync.dma_start(rt[:, :, :, :], xr[l, jg])
                    rts.append(rt)
                # group A: fans F0, F3
                for jg in range(JT // JG):
                    for jj in range(JG):
                        j = jg * JG + jj
                        st, sp = j == 0, j == JT - 1
                        fan_matmuls(rts[jg], jj, st, sp, pf0, 0, 0, 7)
                        fan_matmuls(rts[jg], jj, st, sp, pf3, 3, 4, 3)
                # A copies drain while group B computes
                nc.scalar.copy(gd0[:, :, l], pf0[:, 0:P])
                nc.scalar.mul(g8[:, 0 : 6 * P, l], pf0[:, P : 7 * P], 0.5)
                nc.scalar.mul(g8[:, 6 * P : 9 * P, l], pf3[:, :], 0.5)
                # group B: fans F1, F2
                for jg in range(JT // JG):
                    for jj in range(JG):
                        j = jg * JG + jj
                        st, sp = j == 0, j == JT - 1
                        fan_matmuls(rts[jg], jj, st, sp, pf1, 1, 1, 6)
                        fan_matmuls(rts[jg], jj, st, sp, pf2, 2, 3, 4)
                nc.scalar.copy(gd1[:, :, l], pf1[:, 0:P])
                nc.scalar.mul(g8[:, 9 * P : 14 * P, l], pf1[:, P : 6 * P], 0.5)
                nc.scalar.mul(g8[:, 14 * P : 18 * P, l], pf2[:, :], 0.5)

            # S-stage tail: reduce each stored slot-pair over (b, c) into
            # a [QW, QW] PSUM tile (4 Gram columns per matmul; the host
            # keeps the quad-diagonal [L, L] blocks).  Self pairs run in
            # bf16 quads, cross pairs in fp8 DoubleRow octs.
            for qi in range(NPAIR):
                pt = psSpool.tile([QW, QW], f32, tag="ptS", name="ptS")
                if qi < 2:
                    gsrc = (gd0, gd1)[qi]
                    for ci, c in enumerate(range(0, P, 4)):
                        nc.tensor.matmul(
                            pt[:, :],
                            lhsT=gsrc[:, c : c + 4, :],
                            rhs=gsrc[:, c : c + 4, :],
                            start=(ci == 0),
                            stop=(c + 4 == P),
                        )
                else:
                    base = (qi - 2) * P
                    for ci, c in enumerate(range(0, P, 8)):
                        nc.tensor.matmul(
                            pt[:, :],
                            lhsT=g8[:, base + c : base + c + 8, :].rearrange(
                                "p (i x) l -> p i (x l)", i=2
                            ),
                            rhs=g8[:, base + c : base + c + 8, :].rearrange(
                                "p (i x) l -> p i x l", i=2
                            ),
                            start=(ci == 0),
                            stop=(c + 8 == P),
                            perf_mode=DR,
                        )
                nc.scalar.copy(sS[:, qi * QW : (qi + 1) * QW], pt[:, :])
                if qi == NPAIR // 2 - 1:
                    nc.sync.dma_start(
                        s_out[:, : (qi + 1) * QW], sS[:, : (qi + 1) * QW]
                    )
            nc.sync.dma_start(
                s_out[:, (NPAIR // 2) * QW :], sS[:, (NPAIR // 2) * QW :]
            )

    nc.compile()
    _NC_CACHE["nc"] = nc
    return nc


def _run(activations, trace=False):
    from concourse.bass_utils import run_bass_kernel_spmd

    x = np.asarray(activations, dtype=np.float32)
    assert x.shape == (L, B, D)
    xt_np = np.ascontiguousarray(x.transpose(0, 2, 1)).astype(ml_dtypes.float8_e4m3)
    s_star = xt_np.astype(np.float64).sum(axis=2)  # [L, D], exact sum of fp8 X

    in_maps = []
    for k in range(NCORES):
        cols = np.concatenate(
            [xt_np[:, :, blk * P : (blk + 1) * P] for blk in SIGMA[k]], axis=2
        )  # [L, D, W]
        # pack to the SBUF staging layout: [L, jg, p, jj, i, w] so each
        # (layer, jg) DMA is one fully contiguous block
        packed = np.ascontiguousarray(
            cols.reshape(L, JT // JG, JG, 2, P, W).transpose(0, 1, 4, 2, 3, 5)
        )
        in_maps.append({"xr": packed})
    nc = _build()
    try:
        res = run_bass_kernel_spmd(
            nc, in_maps, core_ids=list(range(NCORES)), trace=trace
        )
    except Exception:
        # transient NRT_EXEC_UNIT_UNRECOVERABLE device states have been
        # observed to clear on the next attempt
        import time

        time.sleep(5)
        res = run_bass_kernel_spmd(
            nc, in_maps, core_ids=list(range(NCORES)), trace=trace
        )

    S = np.zeros((L, L), dtype=np.float64)
    for k in range(NCORES):
        # [QW, NPAIR, QW] -> per pair sum the quad-diagonal [L, L] blocks
        g = res.results[k]["s_out"].astype(np.float64).reshape(4, L, NPAIR, 4, L)
        gd = np.einsum("dicdj->cij", g)  # [NPAIR, L, L]
        S += gd[0] + gd[1]  # self pairs: bf16, weight 1
        for q in range(NQ):
            if _OWNER_W[k][q]:
                S += _OWNER_W[k][q] * gd[2 + q]

    # row-sum statistics are O(L*B*D) -- computed host-side on the same
    # quantized values the device consumed
    xq = xt_np.astype(np.float32)                  # [L, D, B]
    rowsum = np.einsum("ldb,ld->lb", xq, s_star.astype(np.float32))
    T = np.einsum("ib,jb->ij", rowsum, rowsum, dtype=np.float64)
    u = np.einsum("ld,ld->l", s_star, s_star)
    hsic = S - (2.0 / B) * T + np.outer(u, u) / (B * B)
    norms = np.sqrt(np.diagonal(hsic))
    cka = hsic / (norms[:, None] * norms[None, :])
    return cka.astype(np.float32), res


def kernel(activations):
    cka, _ = _run(activations, trace=False)
    return cka


def run_traced(activations):
    return _run(activations, trace=True)


# revision 56
# speedup vs baseline: 1.3366x; 1.1235x over previous
"""Linear-CKA map kernel for Trainium2 (8 NeuronCores, SPMD, no collectives).

Math: for activations X[l] ([B, D] per layer), the reference computes
Gram matrices G_l = X_l X_l^T, double-centers them (Gc = H G H), and
hsic[i,j] = <Gc_i, Gc_j>, cka = hsic / sqrt(diag outer).

We use the expansion (H idempotent, G symmetric):
    hsic_ij = S_ij - (2/B) * T_ij + u_i u_j / B^2
      S_ij = <G_i, G_j>
      rowsum_l[b] = sum_c G_l[b, c] = X_l[b, :] . s_l,  s_l = sum_b X_l[b, :]
      T_ij = sum_b rowsum_i[b] rowsum_j[b]
      u_l  = s_l . s_l

Sharding: the Gram is symmetric over 16x16 blocks of [128, 128]; every
unordered block pair must be co-resident on some core.  A covering
design with 7 column-blocks per core (the information-theoretic floor
at this granularity: 6-block coverings do not exist, by a Fisher /
intersecting-family argument) brings the per-core HBM read down to
L * D * 896 fp8 bytes = 44 MB, vs 63 MB for the baseline 10-block
cyclic cover.  All cores run the SAME program over 7 SBUF "slots"; a
per-core slot permutation SIGMA (found by annealing over an ILP cover)
maps slots to physical blocks so that the fixed slot-pair work list
covers all 136 physical pairs:

  slot-pairs = 2 self pairs (0,0),(1,1) + all 18 cross pairs (a,b)
  with hub a <= 3 -- four "fans" with contiguous partner ranges so
  each PSUM bank hosts exactly one accumulation group:
     F0 = (0 x slots 0..6)   896 cols   2 banks
     F1 = (1 x slots 1..6)   768 cols   2 banks
     F2 = (2 x slots 3..6)   512 cols   1 bank
     F3 = (3 x slots 4..6)   384 cols   1 bank
  The blocks at slots 0,1 across the 8 cores partition all 16 blocks,
  so the two self pairs compute each Gram diagonal block exactly once.

Per layer the fans run as four sequential phase groups (F0, F2, F1,
F3); each group's PSUM banks drain while the following groups compute,
so the next layer's start=True matmuls find their banks free.  pf0 is
layer-double-buffered (the only drain that would otherwise gate the
next layer's first matmul) and the drains are split between ScalarE
(pf0, pf2, pf3) and VectorE (pf1) with all readers of any one PSUM
tile kept on a single engine -- the Tile framework keeps one accessor
chain per tile, so mixed-engine readers would serialize through
cross-engine semaphore hops and pace the whole loop below the DMA
rate.  Gram blocks are stored interleaved [b, c, layer] in fp8
(off-diag at 0.5 scale, diag at 1/16 scale to stay inside e4m3
range), one SBUF tile per (fan, engine) to avoid false WAW chains.
A TensorE tail then reduces each stored slot-pair against itself over
layers ([128b, 2i, 4c x 24l] DoubleRow oct matmuls) into [96, 96]
tiles -- rotating over the freed pf2/pf3/pf1 banks, drained to two
per-engine staging tiles and streamed out in chunks -- whose
quad-diagonal 24x24 blocks the host extracts.  The host de-duplicates
redundantly covered pairs with a precomputed (core, slot-pair)
ownership map and adds the O(L*B*D) row-sum statistics T and u.  No
device collective at all.  TimelineSim: 189.2us (baseline) ->
141.6us; DMA roofline for the 896-column read is ~124us.
"""

import numpy as np
import ml_dtypes

L, B, D = 24, 2048, 2048
NCORES = 8
P = 128
NS = 7                      # column-block slots per core
JT = 8                      # 256-deep DoubleRow contraction tiles
JG = 2                      # j-tiles fetched per rhs DMA
W = NS * P                  # 896 packed columns per core

# fans: (hub slot, first partner slot, #partners); bank-aligned matmul
# splits of each fan's PSUM tile are derived below
FANS = [(0, 0, 7), (1, 1, 6), (2, 3, 4), (3, 4, 3)]
# cross slot-pairs in g8 storage order: F0 partners 1..6, F3, F1
# partners 2..6, F2 (the hub-0/hub-1 self pairs live in gd0/gd1)
CROSSQ = (
    [(0, b) for b in range(1, 7)]
    + [(3, b) for b in range(4, 7)]
    + [(1, b) for b in range(2, 7)]
    + [(2, b) for b in range(3, 7)]
)
NQ = len(CROSSQ)            # 18
NPAIR = NQ + 2              # + the two self pairs
# S-stage processing order: pairs sorted by when their source tile's last
# layer-23 drain lands (g8f2 first, then the diag stores, g8f0, g8f1, and
# g8f3 last), so the S-stage starts right at the end of the Gram loop
QORDER = [1, 16, 17, 18, 19, 0, 2, 3, 4, 5, 6, 7, 11, 12, 13, 14, 15, 8, 9, 10]

# SIGMA[k][s] = physical block held in slot s on core k (annealed so the
# fixed slot-pair list covers all 136 block pairs and slots {0,1}
# partition the 16 diagonal blocks)
SIGMA = [
    [15, 8, 12, 11, 1, 7, 6],
    [2, 11, 14, 8, 9, 10, 3],
    [4, 13, 5, 0, 8, 12, 11],
    [3, 10, 13, 9, 12, 7, 15],
    [0, 7, 6, 5, 10, 3, 9],
    [5, 14, 0, 2, 15, 7, 1],
    [1, 9, 4, 10, 7, 3, 15],
    [12, 6, 13, 4, 1, 2, 14],
]

# host-side dedup: first core covering a physical pair owns it
_OWNER_W = np.zeros((NCORES, NQ), dtype=np.float64)
_seen = {}
for _k in range(NCORES):
    for _q, (_a, _b) in enumerate(CROSSQ):
        _u, _v = SIGMA[_k][_a], SIGMA[_k][_b]
        _pp = (min(_u, _v), max(_u, _v))
        if _pp not in _seen:
            _seen[_pp] = True
            # weight 2 for Gram symmetry x4 to undo the 0.5 fp8 store scale
            _OWNER_W[_k][_q] = 8.0
assert len(_seen) == 120

_NC_CACHE = {}


def _build():
    if "nc" in _NC_CACHE:
        return _NC_CACHE["nc"]
    import concourse.bass as bass
    from concourse import bacc, mybir, tile

    f32 = mybir.dt.float32
    bf16 = mybir.dt.bfloat16
    fp8 = mybir.dt.float8e4
    DR = mybir.MatmulPerfMode.DoubleRow

    nc = bacc.Bacc("TRN2", target_bir_lowering=False, debug=False)

    # host-packed to match the SBUF staging tiles: one fully linear
    # [P, JG, 2, W] block per (layer, jg) DMA
    xr = nc.dram_tensor("xr", [L, JT // JG, P, JG, 2, W], fp8, kind="ExternalInput")
    s_out = nc.dram_tensor("s_out", [4 * L, NPAIR * 4 * L], f32, kind="ExternalOutput")

    QW = 4 * L  # 96: quad/oct S-stage operand width (4 Gram cols x L layers)

    with tile.TileContext(nc) as tc:
        with (
            tc.tile_pool(name="gb", bufs=1) as gbpool,
            tc.tile_pool(name="rt", bufs=16) as rtpool,
            tc.tile_pool(name="psum", bufs=1, space=bass.MemorySpace.PSUM) as pfpool,
            tc.tile_pool(name="psum0", bufs=2, space=bass.MemorySpace.PSUM) as pf0pool,
        ):
            # persistent SBUF Gram store, interleaved [b, c, layer].  One
            # tile per fan destination: Tile tracks WAW at tile granularity,
            # so a single shared store would serialize the ScalarE and
            # VectorE drain chains against each other across layers.
            # diag blocks store fp8 at 1/16 scale (|G_bb| <= ~2370 -> 148,
            # inside even the inf-style e4m3 range); the S-stage then runs
            # DoubleRow octs for every pair, the host undoes the scale
            gd0 = gbpool.tile([P, P, L], fp8, tag="gd0", name="gd0")
            gd1 = gbpool.tile([P, P, L], fp8, tag="gd1", name="gd1")
            g8f0 = gbpool.tile([P, 6 * P, L], fp8, tag="g8f0", name="g8f0")
            g8f1 = gbpool.tile([P, 5 * P, L], fp8, tag="g8f1", name="g8f1")
            g8f2 = gbpool.tile([P, 4 * P, L], fp8, tag="g8f2", name="g8f2")
            g8f3 = gbpool.tile([P, 3 * P, L], fp8, tag="g8f3", name="g8f3")
            # S-stage staging, one per drain engine (same tile-WAW issue);
            # the S accumulators rotate (pf2, pf3, pf1): pf2/pf3 drain on
            # ScalarE into sSa, pf1 on VectorE into sSb
            NSA = NPAIR - NPAIR // 3
            NSB = NPAIR // 3
            sSa = gbpool.tile([QW, NSA * QW], f32, tag="sSa", name="sSa")
            sSb = gbpool.tile([QW, NSB * QW], f32, tag="sSb", name="sSb")

            # one PSUM tile per fan; a fan's matmuls split on its tile's
            # bank boundaries so each bank hosts ONE accumulation group.
            # pf0 is layer-double-buffered (it is the only fan whose drain
            # would otherwise gate the next layer's first matmul); the
            # S-stage later reuses pf2/pf3's banks as its accumulators.
            pf1 = pfpool.tile([P, 6 * P], f32, tag="pf1", name="pf1")
            pf2 = pfpool.tile([P, 4 * P], f32, tag="pf2", name="pf2")
            pf3 = pfpool.tile([P, 3 * P], f32, tag="pf3", name="pf3")

            def fan_matmuls(rt, jj, st, sp, pf, hub, p0, np_):
                lhs = rt[:, jj, :, hub * P : (hub + 1) * P]
                # split the fan's [p0, p0+np_) partner range on the PSUM
                # bank (512 f32) boundaries of pf
                c = 0
                while c < np_ * P:
                    cw = min(512 - c % 512, np_ * P - c)
                    nc.tensor.matmul(
                        pf[:, c : c + cw],
                        lhsT=lhs,
                        rhs=rt[:, jj, :, p0 * P + c : p0 * P + c + cw],
                        start=st,
                        stop=sp,
                        perf_mode=DR,
                    )
                    c += cw

            # per layer the four fans run as four sequential phase groups;
            # each group's PSUM banks drain (ScalarE/VectorE split) while the
            # following three groups compute, so the next layer's start=True
            # on the same banks always finds them free
            # All readers of one PSUM tile stay on ONE engine: Tile keeps a
            # single accessor chain per tile, so mixed-engine readers of the
            # same tile serialize with a cross-engine semaphore hop per
            # reader.  pf0/pf2 drain on ScalarE, pf1/pf3 on VectorE.
            def copies0(l, pf0):
                nc.scalar.mul(gd0[:, :, l], pf0[:, 0:P], 0.0625)
                nc.scalar.mul(g8f0[:, :, l], pf0[:, P : 7 * P], 0.5)

            def copies1(l, pf):
                nc.vector.tensor_scalar_mul(gd1[:, :, l], pf[:, 0:P], 0.0625)
                nc.vector.tensor_scalar_mul(g8f1[:, :, l], pf[:, P : 6 * P], 0.5)

            def copies2(l, pf):
                nc.scalar.mul(g8f2[:, :, l], pf[:, :], 0.5)

            def copies3(l, pf):
                # the last layer's pf3 drain rides VectorE instead: the
                # tail's critical path is the serial ScalarE drain chain of
                # layer L-1 (via the framework's tick waits), and one
                # cross-engine accessor hop on the pf3/g8f3 tiles is cheaper
                # than 505ns of extra chain (measured: applying this to L-2
                # as well lengthens that layer's VectorE chain and loses)
                if l >= L - 1:
                    nc.vector.tensor_scalar_mul(g8f3[:, :, l], pf[:, :], 0.5)
                else:
                    nc.scalar.mul(g8f3[:, :, l], pf[:, :], 0.5)

            for l in range(L):
                rts = []
                for jg in range(JT // JG):
                    rt = rtpool.tile([P, JG, 2, W], fp8, tag="rt", name="rt")
                    nc.sync.dma_start(rt[:, :, :, :], xr[l, jg])
                    rts.append(rt)
                pf0 = pf0pool.tile([P, 7 * P], f32, tag="pf0", name="pf0")
                # group order puts each drain as far as possible ahead of
                # the next layer's reuse of its banks: pf0 (double-buffered,
                # drained lazily after pf2's), then pf2/pf3/pf1 whose bank
                # reuse comes 1-3 groups into the next layer
                groups = (
                    (pf0, 0, 0, 7, None),
                    (pf2, 2, 3, 4, copies2),
                    (pf1, 1, 1, 6, copies1),
                    (pf3, 3, 4, 3, copies3),
                )
                # The last layer splits its j-loop: every group's jg0..jg2
                # matmuls run while the final DMA tile is still in flight,
                # so only the short jg3 chunks (and the drains) remain after
                # the last tile's semaphore fires -- pulling the whole
                # S-stage tail ~3us earlier.  Mid-loop layers keep the
                # group-sequential order that paces the drain pipeline.
                jg_hi = JT // JG if l < L - 1 else JT // JG - 1
                for pf, hub, p0, np_, copies in groups:
                    for jg in range(jg_hi):
                        for jj in range(JG):
                            j = jg * JG + jj
                            fan_matmuls(
                                rts[jg], jj, j == 0, j == JT - 1, pf, hub, p0, np_
                            )
                    if l < L - 1:
                        if copies is not None:
                            copies(l, pf)
                            if pf is pf2:
                                copies0(l, pf0)
                if l == L - 1:
                    for pf, hub, p0, np_, copies in groups:
                        jg = JT // JG - 1
                        for jj in range(JG):
                            j = jg * JG + jj
                            fan_matmuls(
                                rts[jg], jj, False, j == JT - 1, pf, hub, p0, np_
                            )
                        if copies is not None:
                            copies(l, pf)
                            if pf is pf2:
                                copies0(l, pf0)

            # S-stage tail: reduce each stored slot-pair over (b, c) into
            # a [QW, QW] PSUM tile (4 Gram columns per matmul; the host
            # keeps the quad-diagonal [L, L] blocks).  Self pairs run in
            # bf16 quads, cross pairs in fp8 DoubleRow octs.
            # cross pair qi-2 -> (fan tile, local block) in CROSSQ order
            qsrc = (
                [(g8f0, i) for i in range(6)]
                + [(g8f3, i) for i in range(3)]
                + [(g8f1, i) for i in range(5)]
                + [(g8f2, i) for i in range(4)]
            )
            for pos in range(NPAIR):
                qi = QORDER[pos]
                # rotate through the freed pf2/pf3/pf1 banks; pf2/pf3's
                # readers stay on ScalarE and pf1's on VectorE throughout
                pt = (pf2, pf3, pf1)[pos % 3][0:QW, 0:QW]
                if True:
                    if qi < 2:
                        gt, base = (gd0, gd1)[qi], 0
                    else:
                        gt, lq = qsrc[qi - 2]
                        base = lq * P
                    for ci, c in enumerate(range(0, P, 8)):
                        nc.tensor.matmul(
                            pt[:, :],
                            lhsT=gt[:, base + c : base + c + 8, :].rearrange(
                                "p (i x) l -> p i (x l)", i=2
                            ),
                            rhs=gt[:, base + c : base + c + 8, :].rearrange(
                                "p (i x) l -> p i x l", i=2
                            ),
                            start=(ci == 0),
                            stop=(c + 8 == P),
                            perf_mode=DR,
                        )
                # alternate drain engines (each with its own staging tile)
                # so the psS ping-pong round trip halves; stream the export
                # in chunks so the final DMA only covers the last few pairs
                if pos % 3 != 2:
                    h = pos - pos // 3
                    nc.scalar.copy(sSa[:, h * QW : (h + 1) * QW], pt[:, :])
                else:
                    h = pos // 3
                    nc.vector.tensor_copy(sSb[:, h * QW : (h + 1) * QW], pt[:, :])
                if pos == 10:
                    # sSa slots 0..7 are final
                    nc.sync.dma_start(s_out[:, : 8 * QW], sSa[:, : 8 * QW])
                elif pos == 16:
                    # sSa slots 8..11 are final
                    nc.sync.dma_start(
                        s_out[:, 8 * QW : 12 * QW], sSa[:, 8 * QW : 12 * QW]
                    )
                elif pos == 17:
                    # last VectorE pair completes sSb
                    nc.sync.dma_start(s_out[:, NSA * QW :], sSb[:, :])
            nc.sync.dma_start(s_out[:, 12 * QW : NSA * QW], sSa[:, 12 * QW :])

    nc.compile()
    _NC_CACHE["nc"] = nc
    return nc


def _run(activations, trace=False):
    from concourse.bass_utils import run_bass_kernel_spmd

    x = np.asarray(activations, dtype=np.float32)
    assert x.shape == (L, B, D)
    xt_np = np.ascontiguousarray(x.transpose(0, 2, 1)).astype(ml_dtypes.float8_e4m3)
    s_star = xt_np.astype(np.float64).sum(axis=2)  # [L, D], exact sum of fp8 X

    in_maps = []
    for k in range(NCORES):
        cols = np.concatenate(
            [xt_np[:, :, blk * P : (blk + 1) * P] for blk in SIGMA[k]], axis=2
        )  # [L, D, W]
        # pack to the SBUF staging layout: [L, jg, p, jj, i, w] so each
        # (layer, jg) DMA is one fully contiguous block
        packed = np.ascontiguousarray(
            cols.reshape(L, JT // JG, JG, 2, P, W).transpose(0, 1, 4, 2, 3, 5)
        )
        in_maps.append({"xr": packed})
    nc = _build()
    try:
        res = run_bass_kernel_spmd(
            nc, in_maps, core_ids=list(range(NCORES)), trace=trace
        )
    except Exception:
        # transient NRT_EXEC_UNIT_UNRECOVERABLE device states have been
        # observed to clear on the next attempt
        import time

        time.sleep(5)
        res = run_bass_kernel_spmd(
            nc, in_maps, core_ids=list(range(NCORES)), trace=trace
        )

    # export slot layout (positional in QORDER): slots 0..13 = positions
    # with pos % 3 != 2 (ScalarE staging), 14..19 = the rest (VectorE)
    _NSA = NPAIR - NPAIR // 3

    def _slot(qi):
        pos = QORDER.index(qi)
        return pos - pos // 3 if pos % 3 != 2 else _NSA + pos // 3

    S = np.zeros((L, L), dtype=np.float64)
    for k in range(NCORES):
        # [QW, NPAIR, QW] -> per pair sum the quad-diagonal [L, L] blocks
        g = res.results[k]["s_out"].astype(np.float64).reshape(4, L, NPAIR, 4, L)
        gd = np.einsum("dicdj->cij", g)  # [slot, L, L]
        S += 256.0 * (gd[_slot(0)] + gd[_slot(1)])  # self pairs: 1/16 scale
        for q in range(NQ):
            if _OWNER_W[k][q]:
                S += _OWNER_W[k][q] * gd[_slot(2 + q)]

    # row-sum statistics are O(L*B*D) -- computed host-side on the same
    # quantized values the device consumed
    xq = xt_np.astype(np.float32)                  # [L, D, B]
    rowsum = np.einsum("ldb,ld->lb", xq, s_star.astype(np.float32))
    T = np.einsum("ib,jb->ij", rowsum, rowsum, dtype=np.float64)
    u = np.einsum("ld,ld->l", s_star, s_star)
    hsic = S - (2.0 / B) * T + np.outer(u, u) / (B * B)
    norms = np.sqrt(np.diagonal(hsic))
    cka = hsic / (norms[:, None] * norms[None, :])
    return cka.astype(np.float32), res


def kernel(activations):
    cka, _ = _run(activations, trace=False)
    return cka


def run_traced(activations):
    return _run(activations, trace=True)


# revision 60
# speedup vs baseline: 1.3475x; 1.0081x over previous
"""Linear-CKA map kernel for Trainium2 (8 NeuronCores, SPMD, no collectives).

Math: for activations X[l] ([B, D] per layer), the reference computes
Gram matrices G_l = X_l X_l^T, double-centers them (Gc = H G H), and
hsic[i,j] = <Gc_i, Gc_j>, cka = hsic / sqrt(diag outer).

We use the expansion (H idempotent, G symmetric):
    hsic_ij = S_ij - (2/B) * T_ij + u_i u_j / B^2
      S_ij = <G_i, G_j>
      rowsum_l[b] = sum_c G_l[b, c] = X_l[b, :] . s_l,  s_l = sum_b X_l[b, :]
      T_ij = sum_b rowsum_i[b] rowsum_j[b]
      u_l  = s_l . s_l

Sharding: the Gram is symmetric over 16x16 blocks of [128, 128]; every
unordered block pair must be co-resident on some core.  A covering
design with 7 column-blocks per core (the information-theoretic floor
at this granularity: 6-block coverings do not exist, by a Fisher /
intersecting-family argument) brings the per-core HBM read down to
L * D * 896 fp8 bytes = 44 MB, vs 63 MB for the baseline 10-block
cyclic cover.  All cores run the SAME program over 7 SBUF "slots"; a
per-core slot permutation SIGMA (found by annealing over an ILP cover)
maps slots to physical blocks so that the fixed slot-pair work list
covers all 136 physical pairs:

  slot-pairs = 2 self pairs (0,0),(1,1) + all 18 cross pairs (a,b)
  with hub a <= 3 -- four "fans" with contiguous partner ranges so
  each PSUM bank hosts exactly one accumulation group:
     F0 = (0 x slots 0..6)   896 cols   2 banks
     F1 = (1 x slots 1..6)   768 cols   2 banks
     F2 = (2 x slots 3..6)   512 cols   1 bank
     F3 = (3 x slots 4..6)   384 cols   1 bank
  The blocks at slots 0,1 across the 8 cores partition all 16 blocks,
  so the two self pairs compute each Gram diagonal block exactly once.

Per layer the fans run as four sequential phase groups (F0, F2, F1,
F3); each group's PSUM banks drain while the following groups compute,
so the next layer's start=True matmuls find their banks free.  pf0 is
layer-double-buffered (the only drain that would otherwise gate the
next layer's first matmul) and the drains are split between ScalarE
(pf0, pf2, pf3) and VectorE (pf1) with all readers of any one PSUM
tile kept on a single engine -- the Tile framework keeps one accessor
chain per tile, so mixed-engine readers would serialize through
cross-engine semaphore hops and pace the whole loop below the DMA
rate.  Gram blocks are stored interleaved [b, c, layer] in fp8
(off-diag at 0.5 scale, diag at 1/16 scale to stay inside e4m3
range), one SBUF tile per (fan, engine) to avoid false WAW chains.
A TensorE tail then reduces each stored slot-pair against itself over
layers ([128b, 2i, 4c x 24l] DoubleRow oct matmuls) into [96, 96]
tiles -- rotating over the freed pf2/pf3/pf1 banks, drained to two
per-engine staging tiles and streamed out in chunks -- whose
quad-diagonal 24x24 blocks the host extracts.  The host de-duplicates
redundantly covered pairs with a precomputed (core, slot-pair)
ownership map and adds the O(L*B*D) row-sum statistics T and u.  No
device collective at all.  TimelineSim: 189.2us (baseline) ->
141.6us; DMA roofline for the 896-column read is ~124us.
"""

import numpy as np
import ml_dtypes

L, B, D = 24, 2048, 2048
NCORES = 8
P = 128
NS = 7                      # column-block slots per core
JT = 8                      # 256-deep DoubleRow contraction tiles
JG = 2                      # j-tiles fetched per rhs DMA
W = NS * P                  # 896 packed columns per core

# fans: (hub slot, first partner slot, #partners); bank-aligned matmul
# splits of each fan's PSUM tile are derived below
FANS = [(0, 0, 7), (1, 1, 6), (2, 3, 4), (3, 4, 3)]
# cross slot-pairs in g8 storage order: F0 partners 1..6, F3, F1
# partners 2..6, F2 (the hub-0/hub-1 self pairs live in gd0/gd1)
CROSSQ = (
    [(0, b) for b in range(1, 7)]
    + [(3, b) for b in range(4, 7)]
    + [(1, b) for b in range(2, 7)]
    + [(2, b) for b in range(3, 7)]
)
NQ = len(CROSSQ)            # 18
NPAIR = NQ + 2              # + the two self pairs
# S-stage processing order: pairs sorted by when their source tile's last
# layer-23 drain lands (g8f2 first, then the diag stores, g8f0, g8f1, and
# g8f3 last), so the S-stage starts right at the end of the Gram loop
QORDER = [1, 16, 17, 18, 19, 0, 2, 3, 4, 5, 6, 7, 11, 12, 13, 14, 15, 8, 9, 10]

# SIGMA[k][s] = physical block held in slot s on core k (annealed so the
# fixed slot-pair list covers all 136 block pairs and slots {0,1}
# partition the 16 diagonal blocks)
SIGMA = [
    [15, 8, 12, 11, 1, 7, 6],
    [2, 11, 14, 8, 9, 10, 3],
    [4, 13, 5, 0, 8, 12, 11],
    [3, 10, 13, 9, 12, 7, 15],
    [0, 7, 6, 5, 10, 3, 9],
    [5, 14, 0, 2, 15, 7, 1],
    [1, 9, 4, 10, 7, 3, 15],
    [12, 6, 13, 4, 1, 2, 14],
]

# host-side dedup: first core covering a physical pair owns it
_OWNER_W = np.zeros((NCORES, NQ), dtype=np.float64)
_seen = {}
for _k in range(NCORES):
    for _q, (_a, _b) in enumerate(CROSSQ):
        _u, _v = SIGMA[_k][_a], SIGMA[_k][_b]
        _pp = (min(_u, _v), max(_u, _v))
        if _pp not in _seen:
            _seen[_pp] = True
            # weight 2 for Gram symmetry x4 to undo the 0.5 fp8 store scale
            _OWNER_W[_k][_q] = 8.0
assert len(_seen) == 120

_NC_CACHE = {}


def _build():
    if "nc" in _NC_CACHE:
        return _NC_CACHE["nc"]
    import concourse.bass as bass
    from concourse import bacc, mybir, tile

    f32 = mybir.dt.float32
    bf16 = mybir.dt.bfloat16
    fp8 = mybir.dt.float8e4
    DR = mybir.MatmulPerfMode.DoubleRow

    nc = bacc.Bacc("TRN2", target_bir_lowering=False, debug=False)

    # host-packed to match the SBUF staging tiles: one fully linear
    # [P, JG, 2, W] block per (layer, jg) DMA
    xr = nc.dram_tensor("xr", [L, P, JT // JG, JG, 2, W], fp8, kind="ExternalInput")
    s_out = nc.dram_tensor("s_out", [4 * L, NPAIR * 4 * L], f32, kind="ExternalOutput")

    QW = 4 * L  # 96: quad/oct S-stage operand width (4 Gram cols x L layers)

    with tile.TileContext(nc) as tc:
        with (
            tc.tile_pool(name="gb", bufs=1) as gbpool,
            tc.tile_pool(name="rt", bufs=4) as rtpool,
            tc.tile_pool(name="psum", bufs=1, space=bass.MemorySpace.PSUM) as pfpool,
            tc.tile_pool(name="psum0", bufs=2, space=bass.MemorySpace.PSUM) as pf0pool,
        ):
            # persistent SBUF Gram store, interleaved [b, c, layer].  One
            # tile per fan destination: Tile tracks WAW at tile granularity,
            # so a single shared store would serialize the ScalarE and
            # VectorE drain chains against each other across layers.
            # diag blocks store fp8 at 1/16 scale (|G_bb| <= ~2370 -> 148,
            # inside even the inf-style e4m3 range); the S-stage then runs
            # DoubleRow octs for every pair, the host undoes the scale
            gd0 = gbpool.tile([P, P, L], fp8, tag="gd0", name="gd0")
            gd1 = gbpool.tile([P, P, L], fp8, tag="gd1", name="gd1")
            g8f0 = gbpool.tile([P, 6 * P, L], fp8, tag="g8f0", name="g8f0")
            g8f1 = gbpool.tile([P, 5 * P, L], fp8, tag="g8f1", name="g8f1")
            g8f2 = gbpool.tile([P, 4 * P, L], fp8, tag="g8f2", name="g8f2")
            g8f3 = gbpool.tile([P, 3 * P, L], fp8, tag="g8f3", name="g8f3")
            # S-stage staging, one per drain engine (same tile-WAW issue);
            # the S accumulators rotate (pf2, pf3, pf1): pf2/pf3 drain on
            # ScalarE into sSa, pf1 on VectorE into sSb
            NSA = NPAIR - NPAIR // 3
            NSB = NPAIR // 3
            sSa = gbpool.tile([QW, NSA * QW], f32, tag="sSa", name="sSa")
            sSb = gbpool.tile([QW, NSB * QW], f32, tag="sSb", name="sSb")

            # one PSUM tile per fan; a fan's matmuls split on its tile's
            # bank boundaries so each bank hosts ONE accumulation group.
            # pf0 is layer-double-buffered (it is the only fan whose drain
            # would otherwise gate the next layer's first matmul); the
            # S-stage later reuses pf2/pf3's banks as its accumulators.
            pf1 = pfpool.tile([P, 6 * P], f32, tag="pf1", name="pf1")
            pf2 = pfpool.tile([P, 4 * P], f32, tag="pf2", name="pf2")
            pf3 = pfpool.tile([P, 3 * P], f32, tag="pf3", name="pf3")

            def fan_matmuls(rt, jj, st, sp, pf, hub, p0, np_):
                lhs = rt[:, jj, :, hub * P : (hub + 1) * P]
                # split the fan's [p0, p0+np_) partner range on the PSUM
                # bank (512 f32) boundaries of pf
                c = 0
                while c < np_ * P:
                    cw = min(512 - c % 512, np_ * P - c)
                    nc.tensor.matmul(
                        pf[:, c : c + cw],
                        lhsT=lhs,
                        rhs=rt[:, jj, :, p0 * P + c : p0 * P + c + cw],
                        start=st,
                        stop=sp,
                        perf_mode=DR,
                    )
                    c += cw

            # per layer the four fans run as four sequential phase groups;
            # each group's PSUM banks drain (ScalarE/VectorE split) while the
            # following three groups compute, so the next layer's start=True
            # on the same banks always finds them free
            # All readers of one PSUM tile stay on ONE engine: Tile keeps a
            # single accessor chain per tile, so mixed-engine readers of the
            # same tile serialize with a cross-engine semaphore hop per
            # reader.  pf0/pf2 drain on ScalarE, pf1/pf3 on VectorE.
            def copies0(l, pf0):
                nc.scalar.mul(gd0[:, :, l], pf0[:, 0:P], 0.0625)
                nc.scalar.mul(g8f0[:, :, l], pf0[:, P : 7 * P], 0.5)

            def copies1(l, pf):
                nc.vector.tensor_scalar_mul(gd1[:, :, l], pf[:, 0:P], 0.0625)
                nc.vector.tensor_scalar_mul(g8f1[:, :, l], pf[:, P : 6 * P], 0.5)

            def copies2(l, pf):
                nc.scalar.mul(g8f2[:, :, l], pf[:, :], 0.5)

            def copies3(l, pf):
                # the last layer's pf3 drain rides VectorE instead: the
                # tail's critical path is the serial ScalarE drain chain of
                # layer L-1 (via the framework's tick waits), and one
                # cross-engine accessor hop on the pf3/g8f3 tiles is cheaper
                # than 505ns of extra chain (measured: applying this to L-2
                # as well lengthens that layer's VectorE chain and loses)
                if l >= L - 1:
                    nc.vector.tensor_scalar_mul(g8f3[:, :, l], pf[:, :], 0.5)
                else:
                    nc.scalar.mul(g8f3[:, :, l], pf[:, :], 0.5)

            for l in range(L):
                # one whole-layer DMA for all but the last layer (same bytes,
                # 69 fewer issue/semaphore events); layer L-1 keeps per-jg
                # DMAs so its early j-tiles are available as they land
                if l < L - 1:
                    rtf = rtpool.tile(
                        [P, JT // JG, JG, 2, W], fp8, tag="rtL", name="rtf"
                    )
                    nc.sync.dma_start(rtf[:, :, :, :, :], xr[l])
                    rts = [rtf[:, jg] for jg in range(JT // JG)]
                else:
                    rts = []
                    for jg in range(JT // JG):
                        rt = rtpool.tile([P, JG, 2, W], fp8, tag="rt", name="rt")
                        nc.sync.dma_start(rt[:, :, :, :], xr[l, :, jg])
                        rts.append(rt)
                pf0 = pf0pool.tile([P, 7 * P], f32, tag="pf0", name="pf0")
                # group order puts each drain as far as possible ahead of
                # the next layer's reuse of its banks: pf0 (double-buffered,
                # drained lazily after pf2's), then pf2/pf3/pf1 whose bank
                # reuse comes 1-3 groups into the next layer
                groups = (
                    (pf0, 0, 0, 7, None),
                    (pf2, 2, 3, 4, copies2),
                    (pf1, 1, 1, 6, copies1),
                    (pf3, 3, 4, 3, copies3),
                )
                # The last layer splits its j-loop: every group's jg0..jg2
                # matmuls run while the final DMA tile is still in flight,
                # so only the short jg3 chunks (and the drains) remain after
                # the last tile's semaphore fires -- pulling the whole
                # S-stage tail ~3us earlier.  Mid-loop layers keep the
                # group-sequential order that paces the drain pipeline.
                jg_hi = JT // JG if l < L - 1 else JT // JG - 1
                for pf, hub, p0, np_, copies in groups:
                    for jg in range(jg_hi):
                        for jj in range(JG):
                            j = jg * JG + jj
                            fan_matmuls(
                                rts[jg], jj, j == 0, j == JT - 1, pf, hub, p0, np_
                            )
                    if l < L - 1:
                        if copies is not None:
                            copies(l, pf)
                            if pf is pf2:
                                copies0(l, pf0)
                if l == L - 1:
                    for pf, hub, p0, np_, copies in groups:
                        jg = JT // JG - 1
                        for jj in range(JG):
                            j = jg * JG + jj
                            fan_matmuls(
                                rts[jg], jj, False, j == JT - 1, pf, hub, p0, np_
                            )
                        if copies is not None:
                            copies(l, pf)
                            if pf is pf2:
                                copies0(l, pf0)

            # S-stage tail: reduce each stored slot-pair over (b, c) into
            # a [QW, QW] PSUM tile (4 Gram columns per matmul; the host
            # keeps the quad-diagonal [L, L] blocks).  Self pairs run in
            # bf16 quads, cross pairs in fp8 DoubleRow octs.
            # cross pair qi-2 -> (fan tile, local block) in CROSSQ order
            qsrc = (
                [(g8f0, i) for i in range(6)]
                + [(g8f3, i) for i in range(3)]
                + [(g8f1, i) for i in range(5)]
                + [(g8f2, i) for i in range(4)]
            )
            for pos in range(NPAIR):
                qi = QORDER[pos]
                # rotate through the freed pf2/pf3/pf1 banks; pf2/pf3's
                # readers stay on ScalarE and pf1's on VectorE throughout
                pt = (pf2, pf3, pf1)[pos % 3][0:QW, 0:QW]
                if True:
                    if qi < 2:
                        gt, base = (gd0, gd1)[qi], 0
                    else:
                        gt, lq = qsrc[qi - 2]
                        base = lq * P
                    for ci, c in enumerate(range(0, P, 8)):
                        nc.tensor.matmul(
                            pt[:, :],
                            lhsT=gt[:, base + c : base + c + 8, :].rearrange(
                                "p (i x) l -> p i (x l)", i=2
                            ),
                            rhs=gt[:, base + c : base + c + 8, :].rearrange(
                                "p (i x) l -> p i x l", i=2
                            ),
                            start=(ci == 0),
                            stop=(c + 8 == P),
                            perf_mode=DR,
                        )
                # alternate drain engines (each with its own staging tile)
                # so the psS ping-pong round trip halves; stream the export
                # in chunks so the final DMA only covers the last few pairs
                if pos % 3 != 2:
                    h = pos - pos // 3
                    nc.scalar.copy(sSa[:, h * QW : (h + 1) * QW], pt[:, :])
                else:
                    h = pos // 3
                    nc.vector.tensor_copy(sSb[:, h * QW : (h + 1) * QW], pt[:, :])
                if pos == 10:
                    # sSa slots 0..7 are final
                    nc.sync.dma_start(s_out[:, : 8 * QW], sSa[:, : 8 * QW])
                elif pos == 16:
                    # sSa slots 8..11 are final
                    nc.sync.dma_start(
                        s_out[:, 8 * QW : 12 * QW], sSa[:, 8 * QW : 12 * QW]
                    )
                elif pos == 17:
                    # last VectorE pair completes sSb
                    nc.sync.dma_start(s_out[:, NSA * QW :], sSb[:, :])
            nc.sync.dma_start(s_out[:, 12 * QW : NSA * QW], sSa[:, 12 * QW :])

    nc.compile()
    _NC_CACHE["nc"] = nc
    return nc


def _run(activations, trace=False):
    from concourse.bass_utils import run_bass_kernel_spmd

    x = np.asarray(activations, dtype=np.float32)
    assert x.shape == (L, B, D)
    xt_np = np.ascontiguousarray(x.transpose(0, 2, 1)).astype(ml_dtypes.float8_e4m3)
    s_star = xt_np.astype(np.float64).sum(axis=2)  # [L, D], exact sum of fp8 X

    in_maps = []
    for k in range(NCORES):
        cols = np.concatenate(
            [xt_np[:, :, blk * P : (blk + 1) * P] for blk in SIGMA[k]], axis=2
        )  # [L, D, W]
        # pack to the SBUF staging layout: [L, p, jg, jj, i, w] so a
        # whole-layer DMA is one fully contiguous block per partition
        packed = np.ascontiguousarray(
            cols.reshape(L, JT // JG, JG, 2, P, W).transpose(0, 4, 1, 2, 3, 5)
        )
        in_maps.append({"xr": packed})
    nc = _build()
    try:
        res = run_bass_kernel_spmd(
            nc, in_maps, core_ids=list(range(NCORES)), trace=trace
        )
    except Exception:
        # transient NRT_EXEC_UNIT_UNRECOVERABLE device states have been
        # observed to clear on the next attempt
        import time

        time.sleep(5)
        res = run_bass_kernel_spmd(
            nc, in_maps, core_ids=list(range(NCORES)), trace=trace
        )

    # export slot layout (positional in QORDER): slots 0..13 = positions
    # with pos % 3 != 2 (ScalarE staging), 14..19 = the rest (VectorE)
    _NSA = NPAIR - NPAIR // 3

    def _slot(qi):
        pos = QORDER.index(qi)
        return pos - pos // 3 if pos % 3 != 2 else _NSA + pos // 3

    S = np.zeros((L, L), dtype=np.float64)
    for k in range(NCORES):
        # [QW, NPAIR, QW] -> per pair sum the quad-diagonal [L, L] blocks
        g = res.results[k]["s_out"].astype(np.float64).reshape(4, L, NPAIR, 4, L)
        gd = np.einsum("dicdj->cij", g)  # [slot, L, L]
        S += 256.0 * (gd[_slot(0)] + gd[_slot(1)])  # self pairs: 1/16 scale
        for q in range(NQ):
            if _OWNER_W[k][q]:
                S += _OWNER_W[k][q] * gd[_slot(2 + q)]

    # row-sum statistics are O(L*B*D) -- computed host-side on the same
    # quantized values the device consumed
    xq = xt_np.astype(np.float32)                  # [L, D, B]
    rowsum = np.einsum("ldb,ld->lb", xq, s_star.astype(np.float32))
    T = np.einsum("ib,jb->ij", rowsum, rowsum, dtype=np.float64)
    u = np.einsum("ld,ld->l", s_star, s_star)
    hsic = S - (2.0 / B) * T + np.outer(u, u) / (B * B)
    norms = np.sqrt(np.diagonal(hsic))
    cka = hsic / (norms[:, None] * norms[None, :])
    return cka.astype(np.float32), res


def kernel(activations):
    cka, _ = _run(activations, trace=False)
    return cka


def run_traced(activations):
    return _run(activations, trace=True)
